# revision 1
# baseline (speedup 1.0000x reference)
"""Trainium2 Bass kernel: batched 4-point DLT homography (closed-form solve).

Contract: kernel(pts_1_tile, pred_h4p_tile) -> [B, 3, 3] float32, with
B = 524288 split across 8 NeuronCores (batch-parallel, no communication).

Math (per batch element, points p=0..3 with src (x_p,y_p), dst (X_p,Y_p)):
the DLT system rows are
    x h0 + y h1 + h2 = X (1 + x h6 + y h7)
    x h3 + y h4 + h5 = Y (1 + x h6 + y h7)
Eliminating (h0,h1,h2) from the four X-equations via the left null vector n
of M = [(x_p, y_p, 1)] gives one linear equation in (h6,h7); same for the
Y-equations. Solve the 2x2, back out the rest in closed form.

Layout: each core's 65536 elements sit at [128 partitions, 512 free]; every
per-element scalar is a [128, 512] "plane". Planes live at fixed offsets in
slabs so related planes are contiguous and most steps fuse into multi-plane
single instructions (positive-step / broadcast APs only — DVE runs those at
full rate). ScalarE does the interleave<->planar shuffles (with dtype
casts), VectorE + GPSIMD split the elementwise math, greedy-balanced.
Compute planes are fp16 (DVE 2x mode) or fp32; reciprocals and the 2x2
determinant stay fp32 either way.
"""
import sys

for _p in ("/opt/trn_rl_repo", "/root/.axon_site/_ro/trn_rl_repo"):
    if _p not in sys.path:
        sys.path.append(_p)

import numpy as np

import concourse.bass as bass
import concourse.mybir as mybir
from concourse import bacc
from concourse.tile import TileContext
from concourse.bass_utils import run_bass_kernel_spmd

N_CORES = 8
B_TOTAL = 524288
PER_CORE = B_TOTAL // N_CORES  # 65536
PARTS = 128
F = PER_CORE // PARTS  # 512
FP32 = mybir.dt.float32
FP16 = mybir.dt.float16

ADD = mybir.AluOpType.add
SUB = mybir.AluOpType.subtract
MUL = mybir.AluOpType.mult


class _Slab:
    """Bump allocator with explicit free, in F-plane units, first-fit."""

    def __init__(self, nplanes):
        self.free = [(0, nplanes)]
        self.nplanes = nplanes

    def alloc(self, n):
        for idx, (off, ln) in enumerate(self.free):
            if ln >= n:
                if ln == n:
                    self.free.pop(idx)
                else:
                    self.free[idx] = (off + n, ln - n)
                return off
        raise RuntimeError(f"slab OOM: need {n}, free={self.free}")

    def release(self, off, n):
        self.free.append((off, n))
        self.free.sort()
        merged = []
        for o, ln in self.free:
            if merged and merged[-1][0] + merged[-1][1] == o:
                merged[-1] = (merged[-1][0], merged[-1][1] + ln)
            else:
                merged.append([o, ln])
        self.free = [tuple(m) if isinstance(m, list) else m for m in merged]


class _Bal:
    """Greedy VectorE/GPSIMD balance by estimated op time (ns)."""

    def __init__(self, nc, fp16):
        self.nc = nc
        self.fp16 = fp16
        self.t_v = 0.0
        self.t_g = 0.0

    def cv(self, fd, bcast):
        acc = 2.0 if self.fp16 else 1.0
        return (fd / acc + 64.0) / 0.96

    def cg(self, fd):
        return fd * 2.6 / 1.2 + 1400.0

    def pick(self, fd, pin, bcast=False):
        cv, cg = self.cv(fd, bcast), self.cg(fd)
        if pin is None:
            eng = "v" if self.t_v + cv <= self.t_g + cg else "g"
        else:
            eng = pin
        if eng == "v":
            self.t_v += cv
            return self.nc.vector
        self.t_g += cg
        return self.nc.gpsimd


def _fd(ap):
    n = 1
    for d in ap.shape[1:]:
        n *= d
    return n


OPLOG = {}


def _build(nchunk=1, fp16=False):
    OPLOG.clear()
    fc = F // nchunk
    elems = PARTS * fc
    PDT = FP16 if fp16 else FP32

    nc = bacc.Bacc(None, target_bir_lowering=False, debug=True)
    pts = nc.dram_tensor("pts", [PER_CORE, 8], FP32, kind="ExternalInput")
    prd = nc.dram_tensor("prd", [PER_CORE, 8], FP32, kind="ExternalInput")
    out = nc.dram_tensor("out", [PER_CORE, 9], FP32, kind="ExternalOutput")

    # fp32 slab: DMA staging, output staging, recip/det planes
    N32 = 26
    # compute-plane slab (PDT dtype)
    NP = 58

    with TileContext(nc) as tc:
        nb = 1 if nchunk == 1 else 2
        with tc.tile_pool(name="s32", bufs=nb) as pool32, tc.tile_pool(
            name="sp", bufs=nb
        ) as poolp:
            for c in range(nchunk):
                slab32 = pool32.tile([PARTS, N32 * fc], FP32, tag="slab32")
                slabp = poolp.tile([PARTS, NP * fc], PDT, tag="slabp")
                sa32 = _Slab(N32)
                sa = _Slab(NP)
                bal = _Bal(nc, fp16)

                def R32(off, n):
                    return slab32[:, off * fc : (off + n) * fc]

                def R(off, n):
                    return slabp[:, off * fc : (off + n) * fc]

                def V(off, n):
                    return R(off, n).rearrange("p (c f) -> p c f", f=fc)

                def PL(off):
                    return R(off, 1)

                def BC(off, k):
                    return PL(off).unsqueeze(1).broadcast_to((PARTS, k, fc))

                def tt(o, a, b, op, pin=None, bcast=False, desc=""):
                    eng = bal.pick(_fd(o), pin, bcast)
                    ins = eng.tensor_tensor(out=o, in0=a, in1=b, op=op)
                    OPLOG[ins.ins.name] = desc or "tt"

                def stt(o, in0, scalar, in1, op0, op1, desc="stt"):
                    bal.t_v += bal.cv(_fd(o), False)
                    ins = nc.vector.scalar_tensor_tensor(
                        out=o, in0=in0, scalar=scalar, in1=in1, op0=op0, op1=op1
                    )
                    OPLOG[ins.ins.name] = desc

                def scp(o, i, desc="scp"):
                    ins = nc.scalar.copy(out=o, in_=i)
                    OPLOG[ins.ins.name] = desc

                lo = c * elems
                hi = lo + elems

                vt = sa32.alloc(8)
                pt = sa32.alloc(8)
                # split loads into element-halves so compute starts earlier
                half = 4 * fc
                vsrc = pts[lo:hi, :].rearrange("(p f) c -> p (f c)", p=PARTS)
                psrc = prd[lo:hi, :].rearrange("(p f) c -> p (f c)", p=PARTS)
                # vt first: the deint chain (everything except uadd) only
                # needs vt, so pts traffic must not queue behind prd's
                nc.sync.dma_start(out=R32(vt, 8)[:, :half], in_=vsrc[:, :half])
                nc.sync.dma_start(out=R32(vt, 8)[:, half:], in_=vsrc[:, half:])
                nc.sync.dma_start(out=R32(pt, 8)[:, :half], in_=psrc[:, :half])
                nc.sync.dma_start(out=R32(pt, 8)[:, half:], in_=psrc[:, half:])

                # deinterleave (+ cast): comp (0,2,4,6,1,3,5,7) -> planar
                xv = sa.alloc(8)  # [x0,x1,x2,x3,y0,y1,y2,y3]
                uu = sa.alloc(8)  # [X0,X1,X2,X3,Y0,Y1,Y2,Y3]

                # v-deint: two comp-half copies (x-planes first -> diffs start)
                iv = R32(vt, 8).rearrange("p (f c g) -> p g c f", c=4, g=2)
                ov_ = R(xv, 8).rearrange("p (g c f) -> p g c f", c=4, g=2)
                # each deint piece starts as soon as its DMA half lands
                hf = fc // 2
                scp(ov_[:, 0, :, :hf], iv[:, 0, :, :hf], desc="deint_vx1")
                scp(ov_[:, 0, :, hf:], iv[:, 0, :, hf:], desc="deint_vx2")
                scp(ov_[:, 1, :, :hf], iv[:, 1, :, :hf], desc="deint_vy1")
                scp(ov_[:, 1, :, hf:], iv[:, 1, :, hf:], desc="deint_vy2")
                # interleaved u = v + pred (contiguous fp32 full-rate on V),
                # then u-deint copies on ScalarE (which has slack)
                ut = sa32.alloc(8)
                tt(R32(ut, 8)[:, :half], R32(vt, 8)[:, :half],
                   R32(pt, 8)[:, :half], ADD, pin="v", desc="uaddV")
                tt(R32(ut, 8)[:, half:], R32(vt, 8)[:, half:],
                   R32(pt, 8)[:, half:], ADD, pin="g", desc="uaddV2")
                iu = R32(ut, 8).rearrange("p (f c g) -> p g c f", c=4, g=2)
                ou_ = R(uu, 8).rearrange("p (g c f) -> p g c f", c=4, g=2)
                hf = fc // 2
                scp(ou_[:, :, :, :hf], iu[:, :, :, :hf], desc="deint_u1")
                scp(ou_[:, :, :, hf:], iu[:, :, :, hf:], desc="deint_u2")
                sa32.release(vt, 8)
                sa32.release(pt, 8)
                sa32.release(ut, 8)
                ot = sa32.alloc(9)
                # OT is element-interleaved (f*9 + c): out-DMA is contiguous
                ov = R32(ot, 9).rearrange("p (f c) -> p c f", c=9)
                nc.gpsimd.memset(ov[:, 8, :], 1.0)

                # diffs: D = [dx1,dx2,dx3,dy1,dy2,dy3]
                dd = sa.alloc(6)
                xv3 = V(xv, 8)
                tt(V(dd, 6)[:, 0:3, :], xv3[:, 1:4, :], BC(xv, 3), SUB,
                   pin="v", bcast=True, desc="diffx")
                tt(V(dd, 6)[:, 3:6, :], xv3[:, 5:8, :], BC(xv + 4, 3), SUB,
                   pin="v", bcast=True, desc="diffy")
                DX1, DX2, DX3, DY1, DY2, DY3 = range(dd, dd + 6)

                # n: n1=dx2dy3-dx3dy2, n2=dx3dy1-dx1dy3, n3=dx1dy2-dx2dy1
                # six products as three 2-plane ops, pairing the two products
                # that share a factor (the shared one rides the broadcast)
                pab = sa.alloc(6)  # [pa0,pa1,pa2,pb0,pb1,pb2]
                pa, pb = pab, pab + 3
                ddv = V(dd, 6)
                pabv = V(pab, 6)
                tt(pabv[:, 0:5:4, :], ddv[:, 1::-1, :], BC(DY3, 2), MUL,
                   pin="v", bcast=True, desc="pab0")  # (dx2,dx1)*dy3
                tt(pabv[:, 1:6:4, :], ddv[:, 2:0:-1, :], BC(DY1, 2), MUL,
                   pin="v", bcast=True, desc="pab1")  # (dx3,dx2)*dy1
                tt(pabv[:, 2:4:1, :], ddv[:, 0:3:2, :], BC(DY2, 2), MUL,
                   pin="v", bcast=True, desc="pab2")  # (dx1,dx3)*dy2
                ns32 = sa32.alloc(6)  # fp32 [n1,n2,n3,det,h6n,h7n] (one block
                # so a single 2-plane reciprocal covers n3 and det)
                tt(R32(ns32, 3), R(pa, 3), R(pb, 3), SUB, pin="v", desc="ns32sub")
                ns = sa.alloc(4)  # PDT [n0,n1,n2,n3]
                # cast on DVE (contiguous 2D tensor_scalar): keeps the n-chain
                # out of ScalarE's in-order stream behind the other chunk's
                # deinterleaves
                ins = nc.vector.tensor_scalar_add(R(ns + 1, 3), R32(ns32, 3), 0.0)
                OPLOG[ins.ins.name] = "nscast_v"
                bal.t_v += (3 * fc / 2 + 90.0) / 0.96
                t0 = sa.alloc(1)
                tt(PL(t0), PL(ns + 1), PL(ns + 2), ADD, pin="v", desc="t0")
                stt(PL(ns), PL(t0), -1.0, PL(ns + 3), MUL, SUB)  # n0=-(n1+n2)-n3
                sa.release(pab, 6)
                sa.release(t0, 1)

                # dots, grouped by point p: ZW[3p..] = (z_p, z_p x_p, z_p y_p)
                # X- and Y-family products merged: 3 eight-plane ops
                zz = sa.alloc(24)
                zx, zy = zz, zz + 12
                V24 = V(zz, 24)

                def zf(s):
                    return V24[:, s:24:3, :].rearrange(
                        "p (a g) f -> p a g f", a=2
                    )

                nsb = V(ns, 4).unsqueeze(1).broadcast_to((PARTS, 2, 4, fc))
                uu4 = V(uu, 8).rearrange("p (a g) f -> p a g f", a=2)
                xb0 = V(xv, 8)[:, 0:4, :].unsqueeze(1).broadcast_to(
                    (PARTS, 2, 4, fc)
                )
                xb4 = V(xv, 8)[:, 4:8, :].unsqueeze(1).broadcast_to(
                    (PARTS, 2, 4, fc)
                )
                tt(zf(0), nsb, uu4, MUL, pin="v", bcast=True, desc="zzm")
                tt(zf(1), zf(0), xb0, MUL, pin="v", bcast=True, desc="qqm")
                tt(zf(2), zf(0), xb4, MUL, pin="v", bcast=True, desc="rrm")
                tx = sa.alloc(6)
                tt(R(tx, 6), R(zx, 6), R(zx + 6, 6), ADD, pin="v", desc="TX")
                sa.release(zx, 12)
                ty = sa.alloc(6)
                tt(R(ty, 6), R(zy, 6), R(zy + 6, 6), ADD, pin="g", desc="TY")
                sa.release(zy, 12)
                ss = sa.alloc(6)  # [aX,bX,cX,aY,bY,cY]
                tt(R(ss, 3), R(tx, 3), R(tx + 3, 3), ADD, pin="v", desc="ssX")
                tt(R(ss + 3, 3), R(ty, 3), R(ty + 3, 3), ADD, pin="g", desc="ssY")
                sa.release(tx, 6)
                sa.release(ty, 6)

                # 2x2: det = bXcY-bYcX, h6n = cXaY-cYaX, h7n = bYaX-bXaY
                AX, BX, CX, AY, BY, CY = range(ss, ss + 6)
                pcd = sa.alloc(6)  # [pc0,pc1,pc2,pd0,pd1,pd2]
                pc, pd = pcd, pcd + 3
                ssv = V(ss, 6)
                pcdv = V(pcd, 6)
                tt(pcdv[:, 0:5:4, :], ssv[:, 1::-1, :], BC(CY, 2), MUL,
                   pin="v", bcast=True, desc="pcd0")  # (bX,aX)*cY
                tt(pcdv[:, 1:6:4, :], ssv[:, 2:0:-1, :], BC(AY, 2), MUL,
                   pin="v", bcast=True, desc="pcd1")  # (cX,bX)*aY
                tt(pcdv[:, 2:4:1, :], ssv[:, 0:3:2, :], BC(BY, 2), MUL,
                   pin="v", bcast=True, desc="pcd2")  # (aX,cX)*bY
                dt32 = ns32 + 3  # fp32 [det, h6n, h7n] inside the block
                tt(R32(dt32, 3), R(pc, 3), R(pd, 3), SUB, pin="v", desc="dtsub")
                sa.release(pc, 3)
                sa.release(pd, 3)
                sa.release(ss, 6)

                rc32 = sa32.alloc(4)  # [rn3, rdet] out + 2 scratch
                nc.vector.reciprocal_approx_accurate(
                    out=R32(rc32, 2), in_=R32(ns32 + 2, 2), scratch=R32(rc32 + 2, 2)
                )
                bal.t_v += 2 * (2 * fc + 151.0) / 0.96
                h67 = sa.alloc(2)
                # (h6,h7) = (h6n,h7n) * rdet ; mixed fp32 ins -> PDT out
                rdetb = (
                    R32(rc32 + 1, 1).unsqueeze(1).broadcast_to((PARTS, 2, fc))
                )
                tt(V(h67, 2), R32(dt32 + 1, 2).rearrange("p (c f) -> p c f", f=fc),
                   rdetb, MUL, pin="v", bcast=True, desc="h67")
                scp(ov[:, 6:8, :], V(h67, 2))

                # XW_p = X_p (1 + x_p h6 + y_p h7), p=0..2; same for YW
                m1 = sa.alloc(3)
                m2 = sa.alloc(3)
                sp = sa.alloc(3)
                xw = sa.alloc(6)  # [XW0,XW1,XW2,YW0,YW1,YW2]
                tt(V(m1, 3), V(xv, 8)[:, 0:3, :], BC(h67, 3), MUL, pin="v",
                   bcast=True, desc="m1")
                tt(V(m2, 3), V(xv, 8)[:, 4:7, :], BC(h67 + 1, 3), MUL, pin="v",
                   bcast=True, desc="m2")
                tt(R(sp, 3), R(m1, 3), R(m2, 3), ADD, pin="v", desc="sp")
                # w = sp + 1 on DVE tensor_scalar (4x mode, ~260ns) — keeps the
                # serial back-half chain off ScalarE's slow path
                ins = nc.vector.tensor_scalar_add(R(m1, 3), R(sp, 3), 1.0)
                OPLOG[ins.ins.name] = "wp_v"
                bal.t_v += (3 * fc / 4 + 90.0) / 0.96
                tt(V(xw, 6)[:, 0:3, :], V(m1, 3), V(uu, 8)[:, 0:3, :], MUL,
                   pin="v", desc="XW")
                tt(V(xw, 6)[:, 3:6, :], V(m1, 3), V(uu, 8)[:, 4:7, :], MUL,
                   pin="v", desc="YW")
                sa.release(m1, 3)
                sa.release(m2, 3)
                sa.release(sp, 3)
                sa.release(h67, 2)
                sa.release(uu, 8)

                # PQ = (XW1-XW0, XW2-XW0, YW1-YW0, YW2-YW0)
                pq = sa.alloc(4)
                xwv = R(xw, 6).rearrange("p (a b f) -> p a b f", a=2, b=3)
                tt(
                    R(pq, 4).rearrange("p (a b f) -> p a b f", a=2, b=2),
                    xwv[:, :, 1:3, :],
                    xwv[:, :, 0, :].unsqueeze(2).broadcast_to((PARTS, 2, 2, fc)),
                    SUB,
                    pin="v",
                    bcast=True,
                    desc="PQ",
                )

                # rD = 1/n3 came out of the merged reciprocal; hg reads the
                # fp32 value directly (no ScalarE cast hop on the chain)
                sa32.release(ns32, 6)
                sa.release(ns, 4)

                # pE = (P1 dy2, Q1 dy2, dx1 P2, dx1 Q2)
                # pF = (P2 dy1, Q2 dy1, dx2 P1, dx2 Q1)
                pe = sa.alloc(4)
                pf = sa.alloc(4)
                pqv = V(pq, 4)
                tt(V(pe, 4)[:, 0:2, :], pqv[:, 0:3:2, :], BC(DY2, 2), MUL,
                   pin="v", bcast=True, desc="pe01")
                tt(V(pe, 4)[:, 2:4, :], pqv[:, 1:4:2, :], BC(DX1, 2), MUL,
                   pin="v", bcast=True, desc="pe23")
                tt(V(pf, 4)[:, 0:2, :], pqv[:, 1:4:2, :], BC(DY1, 2), MUL,
                   pin="v", bcast=True, desc="pf01")
                tt(V(pf, 4)[:, 2:4, :], pqv[:, 0:3:2, :], BC(DX2, 2), MUL,
                   pin="v", bcast=True, desc="pf23")
                hn = sa.alloc(4)  # [h0n, h3n, h1n, h4n]
                tt(R(hn, 4), R(pe, 4), R(pf, 4), SUB, pin="v", desc="hn")
                hg = sa.alloc(4)  # [h0, h3, h1, h4]
                rdb32 = R32(rc32, 1).unsqueeze(1).broadcast_to((PARTS, 4, fc))
                tt(V(hg, 4), V(hn, 4), rdb32, MUL, pin="v", bcast=True, desc="hg")
                sa.release(pe, 4)
                sa.release(pf, 4)
                sa.release(hn, 4)
                sa.release(pq, 4)
                sa32.release(rc32, 4)

                scp(ov[:, 0:4:3, :], V(hg, 2), desc="hcopy")
                scp(ov[:, 1:5:3, :], V(hg + 2, 2), desc="hcopy")

                # h2 = XW0 - x0 h0 - y0 h1 ; h5 = YW0 - x0 h3 - y0 h4
                ee = sa.alloc(4)  # (x0 h0, y0 h1, x0 h3, y0 h4)
                xy0 = V(xv, 8)[:, 0:5:4, :]  # (x0, y0)
                hgv = V(hg, 4)
                tt(V(ee, 4)[:, 0:2, :], xy0, hgv[:, 0:3:2, :], MUL, pin="v",
                   desc="ee1")
                tt(V(ee, 4)[:, 2:4, :], xy0, hgv[:, 1:4:2, :], MUL, pin="v",
                   desc="ee2")
                s1 = sa.alloc(2)
                eev = V(ee, 4)
                tt(V(s1, 2), V(xw, 6)[:, 0:4:3, :], eev[:, 0:3:2, :], SUB, pin="v", desc="s1")
                # h2,h5 written straight into the strided fp32 output staging
                tt(ov[:, 2:6:3, :], V(s1, 2), eev[:, 1:4:2, :], SUB, pin="v",
                   desc="h25d")
                sa.release(ee, 4)
                sa.release(s1, 2)
                sa.release(hg, 4)
                sa.release(xw, 6)
                sa.release(dd, 6)
                sa.release(xv, 8)

                nc.sync.dma_start(
                    out=out[lo:hi, :].rearrange("(p f) c -> p (f c)", p=PARTS),
                    in_=R32(ot, 9),
                )
                sa32.release(ot, 9)
    nc.finalize()
    return nc


_NC_CACHE = {}


def _get_nc(nchunk=1, fp16=False):
    key = (nchunk, fp16)
    if key not in _NC_CACHE:
        _NC_CACHE[key] = _build(nchunk, fp16)
    return _NC_CACHE[key]


def kernel(pts_1_tile, pred_h4p_tile, _trace=False, _nchunk=2, _fp16=True):
    pts = np.ascontiguousarray(
        np.asarray(pts_1_tile, dtype=np.float32).reshape(B_TOTAL, 8)
    )
    prd = np.ascontiguousarray(
        np.asarray(pred_h4p_tile, dtype=np.float32).reshape(B_TOTAL, 8)
    )
    nc = _get_nc(_nchunk, _fp16)
    in_maps = [
        {
            "pts": pts[i * PER_CORE : (i + 1) * PER_CORE],
            "prd": prd[i * PER_CORE : (i + 1) * PER_CORE],
        }
        for i in range(N_CORES)
    ]
    res = run_bass_kernel_spmd(nc, in_maps, list(range(N_CORES)), trace=_trace)
    outs = np.concatenate([res.results[i]["out"] for i in range(N_CORES)], axis=0)
    H = outs.reshape(B_TOTAL, 3, 3).astype(np.float32)
    if _trace:
        return H, res
    return H



# revision 2
# speedup vs baseline: 1.1185x; 1.1185x over previous
"""Trainium2 Bass kernel: batched 4-point DLT homography (closed-form solve).

Contract: kernel(pts_1_tile, pred_h4p_tile) -> [B, 3, 3] float32, with
B = 524288 split across 8 NeuronCores (batch-parallel, no communication).

Math (per batch element, points p=0..3 with src (x_p,y_p), dst (X_p,Y_p)):
the DLT system rows are
    x h0 + y h1 + h2 = X (1 + x h6 + y h7)
    x h3 + y h4 + h5 = Y (1 + x h6 + y h7)
Eliminating (h0,h1,h2) from the four X-equations via the left null vector n
of M = [(x_p, y_p, 1)] gives one linear equation in (h6,h7); same for the
Y-equations. Solve the 2x2, back out the rest in closed form.

Layout: each core's 65536 elements sit at [128 partitions, 512 free]; every
per-element scalar is a [128, fc] "plane". Two chunks (fc=256) pipeline
DMA-in / compute / DMA-out. All elementwise math runs on DVE (in-order, no
cross-engine stalls on the spine); ScalarE does the interleave<->planar
shuffles with dtype casts; compute planes are fp16 (DVE 2x mode) except the
two reciprocals which run fp32 on a single cast-out pair [n3, det].
Output is [B, 8] (h0..h7); the host appends the constant ninth column.
"""
import sys

for _p in ("/opt/trn_rl_repo", "/root/.axon_site/_ro/trn_rl_repo"):
    if _p not in sys.path:
        sys.path.append(_p)

import numpy as np

import concourse.bass as bass
import concourse.mybir as mybir
from concourse import bacc
from concourse.tile import TileContext
from concourse.bass_utils import run_bass_kernel_spmd

N_CORES = 8
B_TOTAL = 524288
PER_CORE = B_TOTAL // N_CORES  # 65536
PARTS = 128
F = PER_CORE // PARTS  # 512
FP32 = mybir.dt.float32
FP16 = mybir.dt.float16

ADD = mybir.AluOpType.add
SUB = mybir.AluOpType.subtract
MUL = mybir.AluOpType.mult


class _Slab:
    """Bump allocator with explicit free, in F-plane units, first-fit."""

    def __init__(self, nplanes, base=0):
        self.free = [(base, nplanes)]
        self.nplanes = nplanes

    def alloc(self, n):
        for idx, (off, ln) in enumerate(self.free):
            if ln >= n:
                if ln == n:
                    self.free.pop(idx)
                else:
                    self.free[idx] = (off + n, ln - n)
                return off
        raise RuntimeError(f"slab OOM: need {n}, free={self.free}")

    def release(self, off, n):
        self.free.append((off, n))
        self.free.sort()
        merged = []
        for o, ln in self.free:
            if merged and merged[-1][0] + merged[-1][1] == o:
                merged[-1] = (merged[-1][0], merged[-1][1] + ln)
            else:
                merged.append([o, ln])
        self.free = [tuple(m) for m in merged]


def _build(nchunk=2):
    fc = F // nchunk
    elems = PARTS * fc
    hf = fc // 2

    nc = bacc.Bacc(None, target_bir_lowering=False, debug=True)
    pts = nc.dram_tensor("pts", [PER_CORE, 8], FP32, kind="ExternalInput")
    prd = nc.dram_tensor("prd", [PER_CORE, 8], FP32, kind="ExternalInput")
    out = nc.dram_tensor("out", [PER_CORE, 8], FP32, kind="ExternalOutput")

    # fp32 slab: DMA staging (vt,pt), recip block, output staging -- per chunk
    N32C = 8 + 8 + 6 + 8  # 30
    # fp16 compute-plane slab per chunk
    NPC = 64

    with TileContext(nc) as tc:
        with tc.tile_pool(name="s", bufs=1) as pool:
            slab32 = pool.tile([PARTS, N32C * nchunk * fc], FP32, tag="slab32")
            slabp = pool.tile([PARTS, NPC * nchunk * fc], FP16, tag="slabp")

            def R32(off, n):
                return slab32[:, off * fc : (off + n) * fc]

            def R(off, n):
                return slabp[:, off * fc : (off + n) * fc]

            def V(off, n):
                return R(off, n).rearrange("p (c f) -> p c f", f=fc)

            def PL(off):
                return R(off, 1)

            def BC(off, k):
                return PL(off).unsqueeze(1).broadcast_to((PARTS, k, fc))

            def tt(o, a, b, op):
                nc.vector.tensor_tensor(out=o, in0=a, in1=b, op=op)

            def scp(o, i):
                nc.scalar.copy(out=o, in_=i)

            sa32 = [_Slab(N32C, base=c * N32C) for c in range(nchunk)]
            sa = [_Slab(NPC, base=c * NPC) for c in range(nchunk)]

            # ---------- phase 1: input DMA + deinterleave (all chunks) ----
            # per chunk keep (vt, pt, xv, pp) until math phase
            stage = []
            for c in range(nchunk):
                lo = c * elems
                hi = lo + elems
                vt = sa32[c].alloc(8)
                pt = sa32[c].alloc(8)
                half = 4 * fc
                vsrc = pts[lo:hi, :].rearrange("(p f) c -> p (f c)", p=PARTS)
                psrc = prd[lo:hi, :].rearrange("(p f) c -> p (f c)", p=PARTS)
                nc.sync.dma_start(out=R32(vt, 8)[:, :half], in_=vsrc[:, :half])
                nc.sync.dma_start(out=R32(pt, 8)[:, :half], in_=psrc[:, :half])
                nc.sync.dma_start(out=R32(vt, 8)[:, half:], in_=vsrc[:, half:])
                nc.sync.dma_start(out=R32(pt, 8)[:, half:], in_=psrc[:, half:])

                # deinterleave + fp32->fp16 cast on ScalarE, in f-halves so
                # each copy starts as soon as its DMA half lands.
                # comp order per element: (x0,y0,x1,y1,...) -> g=2 is (x|y),
                # c=4 is point index
                xv = sa[c].alloc(8)  # [x0..x3, y0..y3]
                pp = sa[c].alloc(8)  # pred offsets, same order
                iv = R32(vt, 8).rearrange("p (f c g) -> p g c f", c=4, g=2)
                ov_ = R(xv, 8).rearrange("p (g c f) -> p g c f", c=4, g=2)
                ip = R32(pt, 8).rearrange("p (f c g) -> p g c f", c=4, g=2)
                op_ = R(pp, 8).rearrange("p (g c f) -> p g c f", c=4, g=2)
                scp(ov_[:, :, :, :hf], iv[:, :, :, :hf])
                scp(op_[:, :, :, :hf], ip[:, :, :, :hf])
                scp(ov_[:, :, :, hf:], iv[:, :, :, hf:])
                scp(op_[:, :, :, hf:], ip[:, :, :, hf:])
                stage.append((vt, pt, xv, pp))

            # ---------- phase 2: math (DVE spine) + out per chunk ---------
            for c in range(nchunk):
                lo = c * elems
                hi = lo + elems
                vt, pt, xv, pp = stage[c]

                # u = v + p, planar fp16, in halves for earlier start
                uu = sa[c].alloc(8)  # [X0..X3, Y0..Y3]
                tt(V(uu, 8)[:, :, :hf], V(xv, 8)[:, :, :hf],
                   V(pp, 8)[:, :, :hf], ADD)
                tt(V(uu, 8)[:, :, hf:], V(xv, 8)[:, :, hf:],
                   V(pp, 8)[:, :, hf:], ADD)
                sa[c].release(pp, 8)
                sa32[c].release(vt, 8)
                sa32[c].release(pt, 8)

                # diffs: D = [dx1,dx2,dx3,dy1,dy2,dy3]
                dd = sa[c].alloc(6)
                xv3 = V(xv, 8)
                tt(V(dd, 6)[:, 0:3, :], xv3[:, 1:4, :], BC(xv, 3), SUB)
                tt(V(dd, 6)[:, 3:6, :], xv3[:, 5:8, :], BC(xv + 4, 3), SUB)
                DX1, DX2, DX3, DY1, DY2, DY3 = range(dd, dd + 6)

                # null vector: n1=dx2dy3-dx3dy2, n2=dx3dy1-dx1dy3,
                # n3=dx1dy2-dx2dy1; paired products share the broadcast
                pab = sa[c].alloc(6)
                pa, pb = pab, pab + 3
                ddv = V(dd, 6)
                pabv = V(pab, 6)
                tt(pabv[:, 0:5:4, :], ddv[:, 1::-1, :], BC(DY3, 2), MUL)
                tt(pabv[:, 1:6:4, :], ddv[:, 2:0:-1, :], BC(DY1, 2), MUL)
                tt(pabv[:, 2:4:1, :], ddv[:, 0:3:2, :], BC(DY2, 2), MUL)
                # nb = [n0,n1,n2,n3, det,h6n,h7n]; (n3,det) adjacent so one
                # 2-plane cast feeds the merged reciprocal
                nb = sa[c].alloc(7)
                tt(R(nb + 1, 3), R(pa, 3), R(pb, 3), SUB)
                t0 = sa[c].alloc(1)
                tt(PL(t0), PL(nb + 1), PL(nb + 2), ADD)
                nc.vector.scalar_tensor_tensor(
                    out=PL(nb), in0=PL(t0), scalar=-1.0, in1=PL(nb + 3),
                    op0=MUL, op1=SUB,
                )  # n0 = -(n1+n2)-n3
                sa[c].release(pab, 6)
                sa[c].release(t0, 1)

                # quadratic sums: ZW[3p+s] = (n_p U_p, n_p U_p x_p, n_p U_p y_p)
                # X- and Y-family merged into three 8-plane ops
                zz = sa[c].alloc(24)
                zx, zy = zz, zz + 12
                V24 = V(zz, 24)

                def zf(s):
                    return V24[:, s:24:3, :].rearrange(
                        "p (a g) f -> p a g f", a=2
                    )

                nsb = V(nb, 4).unsqueeze(1).broadcast_to((PARTS, 2, 4, fc))
                uu4 = V(uu, 8).rearrange("p (a g) f -> p a g f", a=2)
                xb0 = V(xv, 8)[:, 0:4, :].unsqueeze(1).broadcast_to(
                    (PARTS, 2, 4, fc)
                )
                xb4 = V(xv, 8)[:, 4:8, :].unsqueeze(1).broadcast_to(
                    (PARTS, 2, 4, fc)
                )
                tt(zf(0), nsb, uu4, MUL)
                tt(zf(1), zf(0), xb0, MUL)
                tt(zf(2), zf(0), xb4, MUL)
                tx = sa[c].alloc(6)
                tt(R(tx, 6), R(zx, 6), R(zx + 6, 6), ADD)
                sa[c].release(zx, 12)
                ty = sa[c].alloc(6)
                tt(R(ty, 6), R(zy, 6), R(zy + 6, 6), ADD)
                sa[c].release(zy, 12)
                ss = sa[c].alloc(6)  # [aX,bX,cX,aY,bY,cY]
                tt(R(ss, 3), R(tx, 3), R(tx + 3, 3), ADD)
                tt(R(ss + 3, 3), R(ty, 3), R(ty + 3, 3), ADD)
                sa[c].release(tx, 6)
                sa[c].release(ty, 6)

                # 2x2: det = bXcY-bYcX, h6n = cXaY-cYaX, h7n = bYaX-bXaY
                AX, BX, CX, AY, BY, CY = range(ss, ss + 6)
                pcd = sa[c].alloc(6)
                pc, pd = pcd, pcd + 3
                ssv = V(ss, 6)
                pcdv = V(pcd, 6)
                tt(pcdv[:, 0:5:4, :], ssv[:, 1::-1, :], BC(CY, 2), MUL)
                tt(pcdv[:, 1:6:4, :], ssv[:, 2:0:-1, :], BC(AY, 2), MUL)
                tt(pcdv[:, 2:4:1, :], ssv[:, 0:3:2, :], BC(BY, 2), MUL)
                tt(R(nb + 4, 3), R(pc, 3), R(pd, 3), SUB)  # [det,h6n,h7n]
                sa[c].release(pcd, 6)
                sa[c].release(ss, 6)

                # fp32 reciprocal pair: [1/n3, 1/det]
                f32p = sa32[c].alloc(6)
                nc.vector.tensor_scalar_add(R32(f32p, 2), R(nb + 3, 2), 0.0)
                nc.vector.reciprocal_approx_accurate(
                    out=R32(f32p + 2, 2), in_=R32(f32p, 2),
                    scratch=R32(f32p + 4, 2),
                )
                rc = sa[c].alloc(2)  # fp16 [rn3, rdet]
                nc.vector.tensor_scalar_add(R(rc, 2), R32(f32p + 2, 2), 0.0)
                sa32[c].release(f32p, 6)

                h67 = sa[c].alloc(2)
                tt(V(h67, 2), V(nb + 5, 2), BC(rc + 1, 2), MUL)

                # XW_p = X_p (1 + x_p h6 + y_p h7), p=0..2; same for YW
                m1 = sa[c].alloc(3)
                m2 = sa[c].alloc(3)
                sp = sa[c].alloc(3)
                xw = sa[c].alloc(6)  # [XW0,XW1,XW2,YW0,YW1,YW2]
                tt(V(m1, 3), V(xv, 8)[:, 0:3, :], BC(h67, 3), MUL)
                tt(V(m2, 3), V(xv, 8)[:, 4:7, :], BC(h67 + 1, 3), MUL)
                tt(R(sp, 3), R(m1, 3), R(m2, 3), ADD)
                nc.vector.tensor_scalar_add(R(m1, 3), R(sp, 3), 1.0)  # w, 4x
                tt(V(xw, 6)[:, 0:3, :], V(m1, 3), V(uu, 8)[:, 0:3, :], MUL)
                tt(V(xw, 6)[:, 3:6, :], V(m1, 3), V(uu, 8)[:, 4:7, :], MUL)
                sa[c].release(m1, 3)
                sa[c].release(m2, 3)
                sa[c].release(sp, 3)
                sa[c].release(uu, 8)

                # PQ = (XW1-XW0, XW2-XW0, YW1-YW0, YW2-YW0)
                pq = sa[c].alloc(4)
                xwv = R(xw, 6).rearrange("p (a b f) -> p a b f", a=2, b=3)
                tt(
                    R(pq, 4).rearrange("p (a b f) -> p a b f", a=2, b=2),
                    xwv[:, :, 1:3, :],
                    xwv[:, :, 0, :].unsqueeze(2).broadcast_to(
                        (PARTS, 2, 2, fc)
                    ),
                    SUB,
                )

                # h0 = (P1 dy2 - P2 dy1)/n3 etc.
                pe = sa[c].alloc(4)
                pf = sa[c].alloc(4)
                pqv = V(pq, 4)
                tt(V(pe, 4)[:, 0:2, :], pqv[:, 0:3:2, :], BC(DY2, 2), MUL)
                tt(V(pe, 4)[:, 2:4, :], pqv[:, 1:4:2, :], BC(DX1, 2), MUL)
                tt(V(pf, 4)[:, 0:2, :], pqv[:, 1:4:2, :], BC(DY1, 2), MUL)
                tt(V(pf, 4)[:, 2:4, :], pqv[:, 0:3:2, :], BC(DX2, 2), MUL)
                hn = sa[c].alloc(4)  # [h0n, h3n, h1n, h4n]
                tt(R(hn, 4), R(pe, 4), R(pf, 4), SUB)
                hg = sa[c].alloc(4)  # [h0, h3, h1, h4]
                tt(V(hg, 4), V(hn, 4), BC(rc, 4), MUL)
                sa[c].release(pe, 4)
                sa[c].release(pf, 4)
                sa[c].release(hn, 4)
                sa[c].release(pq, 4)
                sa[c].release(rc, 2)
                sa[c].release(nb, 7)

                # h2 = XW0 - x0 h0 - y0 h1 ; h5 = YW0 - x0 h3 - y0 h4
                ee = sa[c].alloc(4)  # (x0 h0, y0 h1, x0 h3, y0 h4)
                xy0 = V(xv, 8)[:, 0:5:4, :]  # (x0, y0)
                hgv = V(hg, 4)
                tt(V(ee, 4)[:, 0:2, :], xy0, hgv[:, 0:3:2, :], MUL)
                tt(V(ee, 4)[:, 2:4, :], xy0, hgv[:, 1:4:2, :], MUL)
                s1 = sa[c].alloc(2)
                eev = V(ee, 4)
                tt(V(s1, 2), V(xw, 6)[:, 0:4:3, :], eev[:, 0:3:2, :], SUB)
                h25 = sa[c].alloc(2)
                tt(V(h25, 2), V(s1, 2), eev[:, 1:4:2, :], SUB)
                sa[c].release(ee, 4)
                sa[c].release(s1, 2)
                sa[c].release(xw, 6)
                sa[c].release(dd, 6)
                sa[c].release(xv, 8)

                # output staging: element-interleaved [f, 8] fp32 so the
                # out-DMA is contiguous; ScalarE interleaves + casts
                ot = sa32[c].alloc(8)
                ov = R32(ot, 8).rearrange("p (f c) -> p c f", c=8)
                scp(ov[:, 0:4:3, :], V(hg, 2))       # h0, h3
                scp(ov[:, 1:5:3, :], V(hg + 2, 2))   # h1, h4
                scp(ov[:, 2:6:3, :], V(h25, 2))      # h2, h5
                scp(ov[:, 6:8, :], V(h67, 2))        # h6, h7
                sa[c].release(hg, 4)
                sa[c].release(h25, 2)
                sa[c].release(h67, 2)

                nc.sync.dma_start(
                    out=out[lo:hi, :].rearrange("(p f) c -> p (f c)", p=PARTS),
                    in_=R32(ot, 8),
                )
                sa32[c].release(ot, 8)
    nc.finalize()
    return nc


_NC_CACHE = {}


def _get_nc(nchunk=2):
    if nchunk not in _NC_CACHE:
        _NC_CACHE[nchunk] = _build(nchunk)
    return _NC_CACHE[nchunk]


def kernel(pts_1_tile, pred_h4p_tile, _trace=False, _nchunk=2):
    pts = np.ascontiguousarray(
        np.asarray(pts_1_tile, dtype=np.float32).reshape(B_TOTAL, 8)
    )
    prd = np.ascontiguousarray(
        np.asarray(pred_h4p_tile, dtype=np.float32).reshape(B_TOTAL, 8)
    )
    nc = _get_nc(_nchunk)
    in_maps = [
        {
            "pts": pts[i * PER_CORE : (i + 1) * PER_CORE],
            "prd": prd[i * PER_CORE : (i + 1) * PER_CORE],
        }
        for i in range(N_CORES)
    ]
    res = run_bass_kernel_spmd(nc, in_maps, list(range(N_CORES)), trace=_trace)
    H = np.empty((B_TOTAL, 9), dtype=np.float32)
    for i in range(N_CORES):
        H[i * PER_CORE : (i + 1) * PER_CORE, :8] = res.results[i]["out"]
    H[:, 8] = 1.0
    H = H.reshape(B_TOTAL, 3, 3)
    if _trace:
        return H, res
    return H


# revision 6
# speedup vs baseline: 1.1859x; 1.0602x over previous
"""Trainium2 Bass kernel: batched 4-point DLT homography (closed-form solve).

Contract: kernel(pts_1_tile, pred_h4p_tile) -> [B, 3, 3] float32, with
B = 524288 split across 8 NeuronCores (batch-parallel, no communication).

Math (per batch element, points p=0..3 with src (x_p,y_p), dst (X_p,Y_p)):
the DLT system rows are
    x h0 + y h1 + h2 = X (1 + x h6 + y h7)
    x h3 + y h4 + h5 = Y (1 + x h6 + y h7)
Eliminating (h0,h1,h2) from the four X-equations via the left null vector n
of M = [(x_p, y_p, 1)] gives one linear equation in (h6,h7); same for the
Y-equations. Solve the 2x2, back out the rest in closed form.

Layout: each core's 65536 elements sit at [128 partitions, 512 free]; every
per-element scalar is a [128, fc] "plane". Two chunks (fc=256) pipeline
DMA-in / compute / DMA-out. All elementwise math runs on DVE (in-order, no
cross-engine stalls on the spine); ScalarE does the interleave<->planar
shuffles with dtype casts; compute planes are fp16 (DVE 2x mode) except the
two reciprocals which run fp32 on a single cast-out pair [n3, det].
Output is [B, 8] (h0..h7); the host appends the constant ninth column.
"""
import sys

for _p in ("/opt/trn_rl_repo", "/root/.axon_site/_ro/trn_rl_repo"):
    if _p not in sys.path:
        sys.path.append(_p)

import numpy as np

import concourse.bass as bass
import concourse.mybir as mybir
from concourse import bacc
from concourse.tile import TileContext
from concourse.bass_utils import run_bass_kernel_spmd

N_CORES = 8
B_TOTAL = 524288
PER_CORE = B_TOTAL // N_CORES  # 65536
PARTS = 128
F = PER_CORE // PARTS  # 512
FP32 = mybir.dt.float32
FP16 = mybir.dt.float16

ADD = mybir.AluOpType.add
SUB = mybir.AluOpType.subtract
MUL = mybir.AluOpType.mult


class _Slab:
    """Bump allocator with explicit free, in F-plane units, first-fit."""

    def __init__(self, nplanes, base=0):
        self.free = [(base, nplanes)]
        self.nplanes = nplanes

    def alloc(self, n):
        for idx, (off, ln) in enumerate(self.free):
            if ln >= n:
                if ln == n:
                    self.free.pop(idx)
                else:
                    self.free[idx] = (off + n, ln - n)
                return off
        raise RuntimeError(f"slab OOM: need {n}, free={self.free}")

    def release(self, off, n):
        self.free.append((off, n))
        self.free.sort()
        merged = []
        for o, ln in self.free:
            if merged and merged[-1][0] + merged[-1][1] == o:
                merged[-1] = (merged[-1][0], merged[-1][1] + ln)
            else:
                merged.append([o, ln])
        self.free = [tuple(m) for m in merged]


def _build(nchunk=2):
    fc = F // nchunk
    elems = PARTS * fc
    hf = fc // 2

    nc = bacc.Bacc(None, target_bir_lowering=False, debug=True)
    pts = nc.dram_tensor("pts", [PER_CORE, 8], FP32, kind="ExternalInput")
    prd = nc.dram_tensor("prd", [PER_CORE, 8], FP32, kind="ExternalInput")
    out = nc.dram_tensor("out", [PER_CORE, 8], FP32, kind="ExternalOutput")

    # fp32 slab: DMA staging (vt,pt), recip block, output staging -- per chunk
    N32C = 8 + 8 + 6 + 8  # 30
    # fp16 compute-plane slab per chunk
    NPC = 64

    with TileContext(nc) as tc:
        with tc.tile_pool(name="s", bufs=1) as pool:
            slab32 = pool.tile([PARTS, N32C * nchunk * fc], FP32, tag="slab32")
            slabp = pool.tile([PARTS, NPC * nchunk * fc], FP16, tag="slabp")

            def R32(off, n):
                return slab32[:, off * fc : (off + n) * fc]

            def R(off, n):
                return slabp[:, off * fc : (off + n) * fc]

            def V(off, n):
                return R(off, n).rearrange("p (c f) -> p c f", f=fc)

            def PL(off):
                return R(off, 1)

            def BC(off, k):
                return PL(off).unsqueeze(1).broadcast_to((PARTS, k, fc))

            def tt(o, a, b, op):
                nc.vector.tensor_tensor(out=o, in0=a, in1=b, op=op)

            def scp(o, i):
                nc.scalar.copy(out=o, in_=i)

            sa32 = [_Slab(N32C, base=c * N32C) for c in range(nchunk)]
            sa = [_Slab(NPC, base=c * NPC) for c in range(nchunk)]

            # ---------- phase 1: input DMA (+ Scalar deint for chunks>0) --
            # chunk 0's deint runs on DVE (idle at startup) in phase 2;
            # quarter-granular DMA lets it start as soon as data lands.
            # comp order per element: (x0,y0,x1,y1,...) -> g=2 is (x|y),
            # c=4 is point index
            stage = []
            for c in range(nchunk):
                lo = c * elems
                hi = lo + elems
                vt = sa32[c].alloc(8)
                pt = sa32[c].alloc(8)
                vsrc = pts[lo:hi, :].rearrange("(p f) c -> p (f c)", p=PARTS)
                psrc = prd[lo:hi, :].rearrange("(p f) c -> p (f c)", p=PARTS)
                xv = sa[c].alloc(8)  # [x0..x3, y0..y3]
                pp = sa[c].alloc(8)  # pred offsets, same order
                iv = R32(vt, 8).rearrange("p (f c g) -> p g c f", c=4, g=2)
                ov_ = R(xv, 8).rearrange("p (g c f) -> p g c f", c=4, g=2)
                ip = R32(pt, 8).rearrange("p (f c g) -> p g c f", c=4, g=2)
                op_ = R(pp, 8).rearrange("p (g c f) -> p g c f", c=4, g=2)
                if c == 0:
                    q = 2 * fc  # quarter of the interleaved free dim
                    for k in range(4):
                        nc.sync.dma_start(
                            out=R32(vt, 8)[:, k * q : (k + 1) * q],
                            in_=vsrc[:, k * q : (k + 1) * q],
                        )
                        nc.sync.dma_start(
                            out=R32(pt, 8)[:, k * q : (k + 1) * q],
                            in_=psrc[:, k * q : (k + 1) * q],
                        )
                else:
                    half = 4 * fc
                    nc.sync.dma_start(
                        out=R32(vt, 8)[:, :half], in_=vsrc[:, :half]
                    )
                    nc.sync.dma_start(
                        out=R32(pt, 8)[:, :half], in_=psrc[:, :half]
                    )
                    nc.sync.dma_start(
                        out=R32(vt, 8)[:, half:], in_=vsrc[:, half:]
                    )
                    nc.sync.dma_start(
                        out=R32(pt, 8)[:, half:], in_=psrc[:, half:]
                    )
                    scp(ov_[:, :, :, :hf], iv[:, :, :, :hf])
                    scp(op_[:, :, :, :hf], ip[:, :, :, :hf])
                    scp(ov_[:, :, :, hf:], iv[:, :, :, hf:])
                    scp(op_[:, :, :, hf:], ip[:, :, :, hf:])
                stage.append((vt, pt, xv, pp, iv, ov_, ip, op_))

            # ---------- phase 2: math (DVE spine) + out per chunk ---------
            for c in range(nchunk):
                lo = c * elems
                hi = lo + elems
                vt, pt, xv, pp, iv, ov_, ip, op_ = stage[c]

                uu = sa[c].alloc(8)  # [X0..X3, Y0..Y3] = v + p
                if c == 0:
                    # DVE deint (quarters, chasing the DMA) + quarter u-adds
                    qf = fc // 4
                    for k in range(4):
                        s = slice(k * qf, (k + 1) * qf)
                        nc.vector.tensor_scalar_add(
                            ov_[:, :, :, s], iv[:, :, :, s], 0.0
                        )
                        nc.vector.tensor_scalar_add(
                            op_[:, :, :, s], ip[:, :, :, s], 0.0
                        )
                        tt(V(uu, 8)[:, :, s], V(xv, 8)[:, :, s],
                           V(pp, 8)[:, :, s], ADD)
                else:
                    # Scalar did the deint; u-add in halves
                    tt(V(uu, 8)[:, :, :hf], V(xv, 8)[:, :, :hf],
                       V(pp, 8)[:, :, :hf], ADD)
                    tt(V(uu, 8)[:, :, hf:], V(xv, 8)[:, :, hf:],
                       V(pp, 8)[:, :, hf:], ADD)
                sa[c].release(pp, 8)
                sa32[c].release(vt, 8)
                sa32[c].release(pt, 8)

                # diffs: D = [dx1,dx2,dx3,dy1,dy2,dy3]
                dd = sa[c].alloc(6)
                xv3 = V(xv, 8)
                tt(V(dd, 6)[:, 0:3, :], xv3[:, 1:4, :], BC(xv, 3), SUB)
                tt(V(dd, 6)[:, 3:6, :], xv3[:, 5:8, :], BC(xv + 4, 3), SUB)
                DX1, DX2, DX3, DY1, DY2, DY3 = range(dd, dd + 6)

                # null vector: n1=dx2dy3-dx3dy2, n2=dx3dy1-dx1dy3,
                # n3=dx1dy2-dx2dy1; paired products share the broadcast
                pab = sa[c].alloc(6)
                pa, pb = pab, pab + 3
                ddv = V(dd, 6)
                pabv = V(pab, 6)
                tt(pabv[:, 0:5:4, :], ddv[:, 1::-1, :], BC(DY3, 2), MUL)
                tt(pabv[:, 1:6:4, :], ddv[:, 2:0:-1, :], BC(DY1, 2), MUL)
                tt(pabv[:, 2:4:1, :], ddv[:, 0:3:2, :], BC(DY2, 2), MUL)
                # nb = [n0,n1,n2,n3, det,h6n,h7n]; (n3,det) adjacent so one
                # 2-plane cast feeds the merged reciprocal
                nb = sa[c].alloc(7)
                tt(R(nb + 1, 3), R(pa, 3), R(pb, 3), SUB)
                t0 = sa[c].alloc(1)
                tt(PL(t0), PL(nb + 1), PL(nb + 2), ADD)
                nc.vector.scalar_tensor_tensor(
                    out=PL(nb), in0=PL(t0), scalar=-1.0, in1=PL(nb + 3),
                    op0=MUL, op1=SUB,
                )  # n0 = -(n1+n2)-n3
                sa[c].release(pab, 6)
                sa[c].release(t0, 1)

                # quadratic sums: ZW[3p+s] = (n_p U_p, n_p U_p x_p, n_p U_p y_p)
                # X- and Y-family merged into three 8-plane ops
                zz = sa[c].alloc(24)
                zx, zy = zz, zz + 12
                V24 = V(zz, 24)

                def zf(s):
                    return V24[:, s:24:3, :].rearrange(
                        "p (a g) f -> p a g f", a=2
                    )

                nsb = V(nb, 4).unsqueeze(1).broadcast_to((PARTS, 2, 4, fc))
                uu4 = V(uu, 8).rearrange("p (a g) f -> p a g f", a=2)
                xb0 = V(xv, 8)[:, 0:4, :].unsqueeze(1).broadcast_to(
                    (PARTS, 2, 4, fc)
                )
                xb4 = V(xv, 8)[:, 4:8, :].unsqueeze(1).broadcast_to(
                    (PARTS, 2, 4, fc)
                )
                tt(zf(0), nsb, uu4, MUL)
                tt(zf(1), zf(0), xb0, MUL)
                tt(zf(2), zf(0), xb4, MUL)
                tx = sa[c].alloc(6)
                tt(R(tx, 6), R(zx, 6), R(zx + 6, 6), ADD)
                sa[c].release(zx, 12)
                ty = sa[c].alloc(6)
                tt(R(ty, 6), R(zy, 6), R(zy + 6, 6), ADD)
                sa[c].release(zy, 12)
                ss = sa[c].alloc(6)  # [aX,bX,cX,aY,bY,cY]
                tt(R(ss, 3), R(tx, 3), R(tx + 3, 3), ADD)
                tt(R(ss + 3, 3), R(ty, 3), R(ty + 3, 3), ADD)
                sa[c].release(tx, 6)
                sa[c].release(ty, 6)

                # 2x2: det = bXcY-bYcX, h6n = cXaY-cYaX, h7n = bYaX-bXaY
                AX, BX, CX, AY, BY, CY = range(ss, ss + 6)
                pcd = sa[c].alloc(6)
                pc, pd = pcd, pcd + 3
                ssv = V(ss, 6)
                pcdv = V(pcd, 6)
                tt(pcdv[:, 0:5:4, :], ssv[:, 1::-1, :], BC(CY, 2), MUL)
                tt(pcdv[:, 1:6:4, :], ssv[:, 2:0:-1, :], BC(AY, 2), MUL)
                tt(pcdv[:, 2:4:1, :], ssv[:, 0:3:2, :], BC(BY, 2), MUL)
                tt(R(nb + 4, 3), R(pc, 3), R(pd, 3), SUB)  # [det,h6n,h7n]
                sa[c].release(pcd, 6)
                sa[c].release(ss, 6)

                # fp32 reciprocal pair: [1/n3, 1/det]; ~18-bit fast approx is
                # ample for the 2e-2 tolerance given fp16 operands
                f32p = sa32[c].alloc(4)
                nc.vector.tensor_scalar_add(R32(f32p, 2), R(nb + 3, 2), 0.0)
                nc.vector.reciprocal_approx_fast(
                    out=R32(f32p + 2, 2), in_=R32(f32p, 2)
                )
                rc = sa[c].alloc(2)  # fp16 [rn3, rdet]
                nc.vector.tensor_scalar_add(R(rc, 2), R32(f32p + 2, 2), 0.0)
                sa32[c].release(f32p, 4)

                h67 = sa[c].alloc(2)
                tt(V(h67, 2), V(nb + 5, 2), BC(rc + 1, 2), MUL)

                # XW_p = X_p (1 + x_p h6 + y_p h7), p=0..2; same for YW
                m1 = sa[c].alloc(3)
                m2 = sa[c].alloc(3)
                sp = sa[c].alloc(3)
                xw = sa[c].alloc(6)  # [XW0,XW1,XW2,YW0,YW1,YW2]
                tt(V(m1, 3), V(xv, 8)[:, 0:3, :], BC(h67, 3), MUL)
                tt(V(m2, 3), V(xv, 8)[:, 4:7, :], BC(h67 + 1, 3), MUL)
                tt(R(sp, 3), R(m1, 3), R(m2, 3), ADD)
                nc.vector.tensor_scalar_add(R(m1, 3), R(sp, 3), 1.0)  # w, 4x
                tt(V(xw, 6)[:, 0:3, :], V(m1, 3), V(uu, 8)[:, 0:3, :], MUL)
                tt(V(xw, 6)[:, 3:6, :], V(m1, 3), V(uu, 8)[:, 4:7, :], MUL)
                sa[c].release(m1, 3)
                sa[c].release(m2, 3)
                sa[c].release(sp, 3)
                sa[c].release(uu, 8)

                # PQ = (XW1-XW0, XW2-XW0, YW1-YW0, YW2-YW0)
                pq = sa[c].alloc(4)
                xwv = R(xw, 6).rearrange("p (a b f) -> p a b f", a=2, b=3)
                tt(
                    R(pq, 4).rearrange("p (a b f) -> p a b f", a=2, b=2),
                    xwv[:, :, 1:3, :],
                    xwv[:, :, 0, :].unsqueeze(2).broadcast_to(
                        (PARTS, 2, 2, fc)
                    ),
                    SUB,
                )

                # back half: h0..h5 + interleave + out-DMA. The last chunk
                # runs it in f-halves so the output DMA overlaps the final
                # DVE ops instead of trailing them.
                pe = sa[c].alloc(4)
                pf = sa[c].alloc(4)
                hn = sa[c].alloc(4)  # [h0n, h3n, h1n, h4n]
                hg = sa[c].alloc(4)  # [h0, h3, h1, h4]
                ee = sa[c].alloc(4)  # (x0 h0, y0 h1, x0 h3, y0 h4)
                s1 = sa[c].alloc(2)
                h25 = sa[c].alloc(2)
                ot = sa32[c].alloc(8)  # element-interleaved [f, 8] fp32
                ov = R32(ot, 8).rearrange("p (f c) -> p c f", c=8)
                pqv = V(pq, 4)
                xy0 = V(xv, 8)[:, 0:5:4, :]  # (x0, y0)
                hgv = V(hg, 4)
                eev = V(ee, 4)
                odst = out[lo:hi, :].rearrange("(p f) c -> p (f c)", p=PARTS)
                scp(ov[:, 6:8, :], V(h67, 2))  # h6,h7 ready early -> first
                nsplit = 2 if c == nchunk - 1 else 1
                for k in range(nsplit):
                    fs = slice(k * fc // nsplit, (k + 1) * fc // nsplit)
                    # h0 = (P1 dy2 - P2 dy1)/n3 etc.
                    tt(V(pe, 4)[:, 0:2, fs], pqv[:, 0:3:2, fs],
                       BC(DY2, 2)[:, :, fs], MUL)
                    tt(V(pe, 4)[:, 2:4, fs], pqv[:, 1:4:2, fs],
                       BC(DX1, 2)[:, :, fs], MUL)
                    tt(V(pf, 4)[:, 0:2, fs], pqv[:, 1:4:2, fs],
                       BC(DY1, 2)[:, :, fs], MUL)
                    tt(V(pf, 4)[:, 2:4, fs], pqv[:, 0:3:2, fs],
                       BC(DX2, 2)[:, :, fs], MUL)
                    tt(V(hn, 4)[:, :, fs], V(pe, 4)[:, :, fs],
                       V(pf, 4)[:, :, fs], SUB)
                    tt(hgv[:, :, fs], V(hn, 4)[:, :, fs],
                       BC(rc, 4)[:, :, fs], MUL)
                    # h2 = XW0 - x0 h0 - y0 h1 ; h5 = YW0 - x0 h3 - y0 h4
                    tt(eev[:, 0:2, fs], xy0[:, :, fs], hgv[:, 0:3:2, fs], MUL)
                    tt(eev[:, 2:4, fs], xy0[:, :, fs], hgv[:, 1:4:2, fs], MUL)
                    tt(V(s1, 2)[:, :, fs], V(xw, 6)[:, 0:4:3, fs],
                       eev[:, 0:3:2, fs], SUB)
                    tt(V(h25, 2)[:, :, fs], V(s1, 2)[:, :, fs],
                       eev[:, 1:4:2, fs], SUB)
                    scp(ov[:, 0:4:3, fs], hgv[:, 0:2, fs])   # h0, h3
                    scp(ov[:, 1:5:3, fs], hgv[:, 2:4, fs])   # h1, h4
                    scp(ov[:, 2:6:3, fs], V(h25, 2)[:, :, fs])  # h2, h5
                    w = 8 * fc // nsplit
                    nc.sync.dma_start(
                        out=odst[:, k * w : (k + 1) * w],
                        in_=R32(ot, 8)[:, k * w : (k + 1) * w],
                    )
                sa[c].release(pe, 4)
                sa[c].release(pf, 4)
                sa[c].release(hn, 4)
                sa[c].release(pq, 4)
                sa[c].release(rc, 2)
                sa[c].release(nb, 7)
                sa[c].release(ee, 4)
                sa[c].release(s1, 2)
                sa[c].release(xw, 6)
                sa[c].release(dd, 6)
                sa[c].release(xv, 8)
                sa[c].release(hg, 4)
                sa[c].release(h25, 2)
                sa[c].release(h67, 2)
                sa32[c].release(ot, 8)
    nc.finalize()
    return nc


_NC_CACHE = {}


def _get_nc(nchunk=2):
    if nchunk not in _NC_CACHE:
        _NC_CACHE[nchunk] = _build(nchunk)
    return _NC_CACHE[nchunk]


def kernel(pts_1_tile, pred_h4p_tile, _trace=False, _nchunk=2):
    pts = np.ascontiguousarray(
        np.asarray(pts_1_tile, dtype=np.float32).reshape(B_TOTAL, 8)
    )
    prd = np.ascontiguousarray(
        np.asarray(pred_h4p_tile, dtype=np.float32).reshape(B_TOTAL, 8)
    )
    nc = _get_nc(_nchunk)
    in_maps = [
        {
            "pts": pts[i * PER_CORE : (i + 1) * PER_CORE],
            "prd": prd[i * PER_CORE : (i + 1) * PER_CORE],
        }
        for i in range(N_CORES)
    ]
    res = run_bass_kernel_spmd(nc, in_maps, list(range(N_CORES)), trace=_trace)
    H = np.empty((B_TOTAL, 9), dtype=np.float32)
    for i in range(N_CORES):
        H[i * PER_CORE : (i + 1) * PER_CORE, :8] = res.results[i]["out"]
    H[:, 8] = 1.0
    H = H.reshape(B_TOTAL, 3, 3)
    if _trace:
        return H, res
    return H
